# revision 1
# baseline (speedup 1.0000x reference)
"""Trainium2 Bass kernel for nn_DeepESNGatedGRU (2-layer DeepESN + gated GRU).

Strategy (data-parallel over batch, 8 cores, B_local=8):
  Everything on-chip runs in a fully transposed layout (feature dim on SBUF
  partitions, (batch, time) on the free dim) so the sequential recurrences
  never need transposes.

  Per layer, per 64-step chunk:
    projA: winx = x @ Win               (batched matmul over chunk)
    res loop: r = .7 r + .3 tanh(winx_t + r @ Wres)   (sequential, small N)
    projB: zx|rx, cx, gx(+ r @ Wg_r), ptil=tanh(r @ Wp + bp)  (batched)
    GRU loop: only the h-dependent matmuls (Wz_h|Wr_h, Wc_h, Wg_h) are in
      the sequential loop; everything x- and r-dependent was precomputed.

  Matmul inputs are cast to MMDT (bf16/fp16); accumulation in fp32 PSUM;
  carried state kept in fp32 with a low-precision shadow for matmul rhs.
"""
import sys
sys.path.insert(0, '/opt/trn_rl_repo')

import numpy as np

import concourse.bass as bass
import concourse.bacc as bacc
import concourse.mybir as mybir
from concourse.tile import TileContext
from concourse.bass_utils import run_bass_kernel_spmd
from concourse.masks import make_identity

F32 = mybir.dt.float32
MMDT = mybir.dt.float16  # matmul input dtype

B, T, IN, H, R, OUT = 64, 512, 256, 512, 512, 10
NCORES = 8
BL = B // NCORES            # batch per core = 8
C = 64                      # chunk length (timesteps)
NCH = T // C
LEAK = 0.3
P = 128                     # partitions
KH = H // P                 # 4 Ktiles over H/R
AF = mybir.ActivationFunctionType
ALU = mybir.AluOpType


def _w_layer_names(l):
    p = f"L{l}_"
    return [p + n for n in
            ("Win", "Wres", "Wz", "bz", "Wr", "br", "Wc", "bc", "Wg", "bg", "Wp", "bp")]


WEIGHT_NAMES = _w_layer_names(0) + _w_layer_names(1) + ["Wo1", "bo1", "Wo2", "bo2"]


def build_program(T_=T, C_=C):
    """Build the per-core Bass program. T_/C_ overridable for fast tests."""
    NCH_ = T_ // C_
    nc = bacc.Bacc()

    x_in = nc.declare_dram_parameter("x", [BL, T_, IN], MMDT, isOutput=False)
    w = {}
    shapes = {}
    for l, lin in ((0, IN), (1, H)):
        p = f"L{l}_"
        shapes[p + "Win"] = [lin, R]
        shapes[p + "Wres"] = [R, R]
        for g in "zrc":
            shapes[p + "W" + g] = [lin + H, H]
            shapes[p + "b" + g] = [H]
        shapes[p + "Wg"] = [lin + H + R, H]
        shapes[p + "bg"] = [H]
        shapes[p + "Wp"] = [R, H]
        shapes[p + "bp"] = [H]
    shapes["Wo1"] = [H, H]
    shapes["bo1"] = [H]
    shapes["Wo2"] = [H, OUT]
    shapes["bo2"] = [OUT]
    for name, shp in shapes.items():
        # weights are pre-cast to fp16 on the host so the device DMA is a
        # straight HWDGE copy (the casting SWDGE path on gpsimd runs at Q7
        # software speed and dominates the whole kernel); biases stay fp32
        w[name] = nc.declare_dram_parameter(
            name, shp, F32 if name.split("_")[-1].startswith("b") else MMDT,
            isOutput=False)
    out_ext = nc.declare_dram_parameter("out", [OUT, BL], F32, isOutput=True)

    with TileContext(nc) as tc:
        with tc.tile_pool(name="persist", bufs=1) as PERS:
            ident = PERS.tile([P, P], F32, tag="ident")
            make_identity(nc, ident[:])
            ident16 = PERS.tile([P, P], MMDT, tag="ident16")
            nc.vector.tensor_copy(ident16[:], ident[:])
            # L0 input, transposed: [k(2), b(BL), t(T_)]
            xT = PERS.tile([P, 2 * BL * T_], MMDT, tag="xT")
            # L0 output (= L1 input), transposed: [k(4), b, t]
            h0sb = PERS.tile([P, KH * BL * T_], MMDT, tag="h0sb")
            # final hidden state (for the head, outlives layer pools)
            hfin = PERS.tile([P, KH * BL], MMDT, tag="hfin")

            # ---- load + transpose x ----
            x_flat = x_in.rearrange("b t i -> (b t) i")
            nrow = BL * T_ // P
            with tc.tile_pool(name="xload", bufs=3) as XL, \
                 tc.tile_pool(name="xps", bufs=4, space="PSUM") as XP:
                for i in range(nrow):
                    xt_nat = XL.tile([P, IN], MMDT, tag="xnat")
                    nc.sync.dma_start(out=xt_nat[:], in_=x_flat[i * P:(i + 1) * P, :])
                    for k in range(IN // P):
                        ps = XP.tile([P, P], MMDT, tag="xtp")
                        nc.tensor.transpose(ps[:], xt_nat[:, k * P:(k + 1) * P], ident16[:])
                        nc.vector.tensor_copy(
                            xT[:, k * BL * T_ + i * P: k * BL * T_ + (i + 1) * P], ps[:])

            def xT_rhs(k, c):
                """rhs Ktile slice for chunk c: free dims (b, tau)."""
                a = xT[:, k * BL * T_:(k + 1) * BL * T_]
                a = a.rearrange("p (b t) -> p b t", b=BL)
                return a[:, :, c * C_:(c + 1) * C_]

            def h0_rhs(k, c):
                a = h0sb[:, k * BL * T_:(k + 1) * BL * T_]
                a = a.rearrange("p (b t) -> p b t", b=BL)
                return a[:, :, c * C_:(c + 1) * C_]

            emit_layer(nc, tc, 0, IN, xT_rhs, w, h0sb, None, T_, C_, NCH_, ident16)
            emit_layer(nc, tc, 1, H, h0_rhs, w, None, hfin, T_, C_, NCH_, ident16)

            # ---- output head ----
            emit_head(nc, tc, w, hfin, out_ext)

    nc.compile()
    return nc


def load_w_tiles(nc, pool, src, row0, nk, nm, name):
    """Load lhsT tiles from DRAM matrix src rows [row0:row0+nk*128] into an
    SBUF tile laid out (128, nk*nm*128) with col index (k*nm+m)*128, cast to MMDT."""
    t = pool.tile([P, nk * nm * P], MMDT, name=name, tag=name)
    for k in range(nk):
        nc.sync.dma_start(
            out=t[:, k * nm * P:(k + 1) * nm * P],
            in_=src[row0 + k * P: row0 + (k + 1) * P, :])
    return t


def load_bias(nc, pool, src, nm, name):
    t = pool.tile([P, nm], F32, name=name, tag=name)
    nc.sync.dma_start(out=t[:], in_=src.rearrange("(m p) -> p m", p=P))
    return t


def emit_layer(nc, tc, l, lin, in_rhs, w, hout_sb, hfin, T_, C_, NCH_, ident16):
    """Emit one ESN+GRU layer. in_rhs(k, c) gives the transposed input slice.
    hout_sb: SBUF tile to write transposed outputs into (or None for layer 1).
    hfin: persistent tile to copy the final h_bf into (layer 1 only)."""
    pfx = f"L{l}_"
    KL = lin // P
    with tc.tile_pool(name=f"wts{l}", bufs=1) as WP:
        # recurrent weights (h-parts); zr packed: m 0..3 = z, 4..7 = r
        wzr_h = WP.tile([P, KH * 8 * P], MMDT, name=f"wzrh{l}", tag=f"wzrh{l}")
        for k in range(KH):
            nc.sync.dma_start(out=wzr_h[:, (k * 8) * P:(k * 8 + 4) * P],
                                in_=w[pfx + "Wz"][lin + k * P: lin + (k + 1) * P, :])
            nc.sync.dma_start(out=wzr_h[:, (k * 8 + 4) * P:(k * 8 + 8) * P],
                                in_=w[pfx + "Wr"][lin + k * P: lin + (k + 1) * P, :])
        wc_h = load_w_tiles(nc, WP, w[pfx + "Wc"], lin, KH, KH, f"wch{l}")
        wg_h = load_w_tiles(nc, WP, w[pfx + "Wg"], lin, KH, KH, f"wgh{l}")
        wres = load_w_tiles(nc, WP, w[pfx + "Wres"], 0, KH, KH, f"wres{l}")
        wp_r = load_w_tiles(nc, WP, w[pfx + "Wp"], 0, KH, KH, f"wp{l}")
        wg_r = load_w_tiles(nc, WP, w[pfx + "Wg"], lin + H, KH, KH, f"wgr{l}")
        win_x = load_w_tiles(nc, WP, w[pfx + "Win"], 0, KL, KH, f"winx{l}")
        # x-part weights; zr packed like wzr_h
        wzr_x = WP.tile([P, KL * 8 * P], MMDT, name=f"wzrx{l}", tag=f"wzrx{l}")
        for k in range(KL):
            nc.sync.dma_start(out=wzr_x[:, (k * 8) * P:(k * 8 + 4) * P],
                                in_=w[pfx + "Wz"][k * P:(k + 1) * P, :])
            nc.sync.dma_start(out=wzr_x[:, (k * 8 + 4) * P:(k * 8 + 8) * P],
                                in_=w[pfx + "Wr"][k * P:(k + 1) * P, :])
        wc_x = load_w_tiles(nc, WP, w[pfx + "Wc"], 0, KL, KH, f"wcx{l}")
        wg_x = load_w_tiles(nc, WP, w[pfx + "Wg"], 0, KL, KH, f"wgx{l}")
        bzr = WP.tile([P, 8], F32, name=f"bzr{l}", tag=f"bzr{l}")
        nc.sync.dma_start(out=bzr[:, 0:4], in_=w[pfx + "bz"].rearrange("(m p) -> p m", p=P))
        nc.sync.dma_start(out=bzr[:, 4:8], in_=w[pfx + "br"].rearrange("(m p) -> p m", p=P))
        bc = load_bias(nc, WP, w[pfx + "bc"], KH, f"bc{l}")
        bg = load_bias(nc, WP, w[pfx + "bg"], KH, f"bg{l}")
        bp = load_bias(nc, WP, w[pfx + "bp"], KH, f"bp{l}")

        CB = BL * C_  # rows per chunk = 512
        with tc.tile_pool(name=f"state{l}", bufs=4) as SP, \
             tc.tile_pool(name=f"chunk{l}", bufs=2) as CH, \
             tc.tile_pool(name=f"step{l}", bufs=4) as ST, \
             tc.tile_pool(name=f"ps{l}", bufs=1, space="PSUM") as PS:

            # carried state (fp16 only; fp32 accumulation lives in PSUM)
            r0 = SP.tile([P, KH * BL], MMDT, tag="rs", name=f"r0_{l}")
            h0 = SP.tile([P, KH * BL], MMDT, tag="hs", name=f"h0_{l}")
            nc.vector.memset(r0[:], 0.0)
            nc.vector.memset(h0[:], 0.0)
            state = {'r': r0[:], 'h': h0[:]}

            chunk_tiles = {}

            def chunk_prep(c):
                """Generator emitting projA + reservoir loop + projB for chunk c
                in pieces, so emission can interleave with chunk c-1's GRU."""
                ct = {
                    'winx': CH.tile([P, KH * CB], MMDT, tag="winx", name=f"winx{l}_{c}"),
                    'zrx': CH.tile([P, 8 * CB], MMDT, tag="zrx", name=f"zrx{l}_{c}"),
                    'cx': CH.tile([P, KH * CB], MMDT, tag="cx", name=f"cx{l}_{c}"),
                    'gx': CH.tile([P, KH * CB], MMDT, tag="gx", name=f"gx{l}_{c}"),
                    'ptil': CH.tile([P, KH * CB], MMDT, tag="ptil", name=f"ptil{l}_{c}"),
                    'rchunk': CH.tile([P, KH * CB], MMDT, tag="rchunk", name=f"rch{l}_{c}"),
                }
                chunk_tiles[c] = ct
                winx, rchunk = ct['winx'], ct['rchunk']
                # chunk tiles are laid out (tau, m, b) so per-step slices are
                # contiguous; proj writes below use a (b-outer, t-inner)
                # strided AP matching the PSUM's (b, tau) column order.
                def tmb(tile, m, nm):
                    return tile.rearrange("p (t m b) -> p m b t", m=nm, b=BL)[:, m, :, :]
                # ---- projA: winx (no bias) ----
                for m in range(KH):
                    ps = PS.tile([P, CB], F32, tag="pj", bufs=2, name=f"pjA{l}_{c}_{m}")
                    for k in range(KL):
                        nc.tensor.matmul(ps[:], win_x[:, (k * KH + m) * P:(k * KH + m + 1) * P],
                                         in_rhs(k, c), start=(k == 0), stop=(k == KL - 1))
                        yield
                    nc.scalar.copy(tmb(winx, m, KH), ps[:])
                    yield
                # ---- reservoir loop ----
                # s-state trick: s = r/LEAK with Wres/Wp/Wg_r pre-scaled by
                # LEAK on the host, so s' = (1-LEAK)*s + tanh(x Win + s W~res)
                # exactly -- one stt op, written straight into the rchunk
                # slice (strided (k,b) at fixed tau) which doubles as state.
                # (k b) collapses to one stride-C_ dim since k's stride is
                # exactly BL*C_ -- per-step slices stay 2-dim [P, 32]
                rch2 = rchunk.rearrange("p (kb t) -> p kb t", t=C_)
                for tau in range(C_):
                    ps = PS.tile([P, KH * BL], F32, tag="res", bufs=2, name=f"res{l}_{c}_{tau}")
                    r_st = state['r']
                    nc.tensor.matmul(ps[:], ident16[:],
                                     winx[:, tau * KH * BL:(tau + 1) * KH * BL],
                                     start=True, stop=False)
                    for m in range(KH):
                        for k in range(KH):
                            nc.tensor.matmul(
                                ps[:, m * BL:(m + 1) * BL],
                                wres[:, (k * KH + m) * P:(k * KH + m + 1) * P],
                                r_st[:, k * BL:(k + 1) * BL],
                                start=False, stop=(k == KH - 1))
                    tt = ST.tile([P, KH * BL], F32, tag="tt")
                    nc.scalar.activation(tt[:], ps[:], AF.Tanh)
                    r_new = SP.tile([P, KH * BL], MMDT, tag="rs")
                    nc.vector.scalar_tensor_tensor(
                        r_new[:], r_st, 1.0 - LEAK, tt[:], ALU.mult, ALU.add)
                    nc.scalar.copy(rch2[:, :, tau], r_new[:])
                    state['r'] = r_new[:]
                    yield
                # ---- projB ----
                for m in range(8):
                    ps = PS.tile([P, CB], F32, tag="pj", bufs=2, name=f"pjZ{l}_{c}_{m}")
                    for k in range(KL):
                        nc.tensor.matmul(ps[:], wzr_x[:, (k * 8 + m) * P:(k * 8 + m + 1) * P],
                                         in_rhs(k, c), start=(k == 0), stop=(k == KL - 1))
                        yield
                    nc.scalar.activation(tmb(ct['zrx'], m, 8), ps[:],
                                         AF.Identity, bias=bzr[:, m:m + 1])
                    yield
                for m in range(KH):
                    ps = PS.tile([P, CB], F32, tag="pj", bufs=2, name=f"pjC{l}_{c}_{m}")
                    for k in range(KL):
                        nc.tensor.matmul(ps[:], wc_x[:, (k * KH + m) * P:(k * KH + m + 1) * P],
                                         in_rhs(k, c), start=(k == 0), stop=(k == KL - 1))
                        yield
                    nc.scalar.activation(tmb(ct['cx'], m, KH), ps[:],
                                         AF.Identity, bias=bc[:, m:m + 1])
                    yield
                for m in range(KH):
                    ps = PS.tile([P, CB], F32, tag="pj", bufs=2, name=f"pjG{l}_{c}_{m}")
                    for k in range(KL):
                        nc.tensor.matmul(ps[:], wg_x[:, (k * KH + m) * P:(k * KH + m + 1) * P],
                                         in_rhs(k, c), start=(k == 0), stop=False)
                        yield
                    for k in range(KH):
                        nc.tensor.matmul(ps[:], wg_r[:, (k * KH + m) * P:(k * KH + m + 1) * P],
                                         rchunk[:, k * CB:(k + 1) * CB],
                                         start=False, stop=(k == KH - 1))
                        yield
                    nc.scalar.activation(tmb(ct['gx'], m, KH), ps[:],
                                         AF.Identity, bias=bg[:, m:m + 1])
                    yield
                for m in range(KH):
                    ps = PS.tile([P, CB], F32, tag="pj", bufs=2, name=f"pjP{l}_{c}_{m}")
                    for k in range(KH):
                        nc.tensor.matmul(ps[:], wp_r[:, (k * KH + m) * P:(k * KH + m + 1) * P],
                                         rchunk[:, k * CB:(k + 1) * CB],
                                         start=(k == 0), stop=(k == KH - 1))
                        yield
                    nc.scalar.activation(tmb(ct['ptil'], m, KH), ps[:],
                                         AF.Tanh, bias=bp[:, m:m + 1])
                    yield

            def gru_step(c, tau, adv=lambda: None):
                ct = chunk_tiles[c]
                h_st = state['h']
                NB = KH * BL
                # x-projections (biases folded) are injected into PSUM by an
                # identity matmul (start=True), so the gates read PSUM directly.
                # reset-gate Mtiles first so Wc's input is ready while the
                # z Mtiles still stream through the PE.
                ps_r = PS.tile([P, NB], F32, tag="pr", name=f"pr{l}_{c}_{tau}")
                nc.tensor.matmul(ps_r[:], ident16[:],
                                 ct['zrx'][:, tau * 2 * NB + NB:(tau + 1) * 2 * NB],
                                 start=True, stop=False)
                for m in range(KH):
                    for k in range(KH):
                        nc.tensor.matmul(
                            ps_r[:, m * BL:(m + 1) * BL],
                            wzr_h[:, (k * 8 + 4 + m) * P:(k * 8 + 4 + m + 1) * P],
                            h_st[:, k * BL:(k + 1) * BL],
                            start=False, stop=(k == KH - 1))
                ps_z = PS.tile([P, NB], F32, tag="pz", name=f"pz{l}_{c}_{tau}")
                nc.tensor.matmul(ps_z[:], ident16[:],
                                 ct['zrx'][:, tau * 2 * NB:tau * 2 * NB + NB],
                                 start=True, stop=False)
                for m in range(KH):
                    for k in range(KH):
                        nc.tensor.matmul(
                            ps_z[:, m * BL:(m + 1) * BL],
                            wzr_h[:, (k * 8 + m) * P:(k * 8 + m + 1) * P],
                            h_st[:, k * BL:(k + 1) * BL],
                            start=False, stop=(k == KH - 1))
                reset = ST.tile([P, NB], F32, tag="reset")
                nc.scalar.activation(reset[:], ps_r[:], AF.Sigmoid)
                rh = ST.tile([P, NB], MMDT, tag="rh")
                nc.vector.tensor_tensor(rh[:], reset[:], h_st[:, :], ALU.mult)

                adv()  # fill the PE wait-for-rh gap with companion work
                ps_c = PS.tile([P, NB], F32, tag="pc", name=f"pc{l}_{c}_{tau}")
                nc.tensor.matmul(ps_c[:], ident16[:],
                                 ct['cx'][:, tau * NB:(tau + 1) * NB],
                                 start=True, stop=False)
                for m in range(KH):
                    for k in range(KH):
                        nc.tensor.matmul(
                            ps_c[:, m * BL:(m + 1) * BL],
                            wc_h[:, (k * KH + m) * P:(k * KH + m + 1) * P],
                            rh[:, k * BL:(k + 1) * BL],
                            start=False, stop=(k == KH - 1))
                zz = ST.tile([P, NB], F32, tag="zz")
                nc.scalar.activation(zz[:], ps_z[:], AF.Sigmoid)
                cc = ST.tile([P, NB], F32, tag="cc")
                nc.scalar.activation(cc[:], ps_c[:], AF.Tanh)
                d = ST.tile([P, NB], F32, tag="d")
                nc.vector.tensor_tensor(d[:], cc[:], h_st[:, :], ALU.subtract)
                e = ST.tile([P, NB], F32, tag="e")
                nc.vector.tensor_tensor(e[:], zz[:], d[:], ALU.mult)
                hg = ST.tile([P, NB], MMDT, tag="hg")
                nc.vector.tensor_tensor(hg[:], h_st[:, :], e[:], ALU.add)

                adv()  # fill the PE wait-for-hg gap
                ps_g = PS.tile([P, NB], F32, tag="pg", name=f"pg{l}_{c}_{tau}")
                nc.tensor.matmul(ps_g[:], ident16[:],
                                 ct['gx'][:, tau * NB:(tau + 1) * NB],
                                 start=True, stop=False)
                for m in range(KH):
                    for k in range(KH):
                        nc.tensor.matmul(
                            ps_g[:, m * BL:(m + 1) * BL],
                            wg_h[:, (k * KH + m) * P:(k * KH + m + 1) * P],
                            hg[:, k * BL:(k + 1) * BL],
                            start=False, stop=(k == KH - 1))
                gg = ST.tile([P, NB], F32, tag="gg")
                nc.scalar.activation(gg[:], ps_g[:], AF.Sigmoid)
                d2 = ST.tile([P, NB], F32, tag="d2")
                nc.vector.tensor_tensor(d2[:], ct['ptil'][:, tau * NB:(tau + 1) * NB],
                                        hg[:], ALU.subtract)
                e2 = ST.tile([P, NB], F32, tag="e2")
                nc.vector.tensor_tensor(e2[:], gg[:], d2[:], ALU.mult)
                hn = SP.tile([P, NB], MMDT, tag="hs")
                nc.vector.tensor_tensor(hn[:], hg[:], e2[:], ALU.add)
                if hout_sb is not None:
                    hck = hout_sb.rearrange(
                        "p (kb t) -> p kb t", t=T_)[:, :, c * C_ + tau]
                    nc.scalar.copy(hck, hn[:])
                state['h'] = hn[:]

            # software pipeline: chunk c's GRU overlaps chunk c+1's prep.
            # Companion pieces are advanced at the PE's intra-step chain-wait
            # points (the engine queue is FIFO, so only already-emitted work
            # can fill a gap).
            for _ in chunk_prep(0):
                pass
            # pieces per chunk_prep: projA KH*(KL+1) + res C_ + projB
            n_pieces = KH * (KL + 1) + C_ + 8 * (KL + 1) + KH * (KL + 1) \
                + KH * (KL + KH + 1) + KH * (KH + 1)
            for c in range(NCH_):
                box = {'gen': chunk_prep(c + 1) if c + 1 < NCH_ else None,
                       'deficit': 0.0}
                per_point = n_pieces / (3.0 * C_)

                def adv():
                    if box['gen'] is None:
                        return
                    box['deficit'] += per_point
                    while box['deficit'] >= 1.0:
                        try:
                            next(box['gen'])
                        except StopIteration:
                            box['gen'] = None
                            return
                        box['deficit'] -= 1.0

                for tau in range(C_):
                    gru_step(c, tau, adv)
                    adv()
                while box['gen'] is not None:
                    try:
                        next(box['gen'])
                    except StopIteration:
                        box['gen'] = None
                del chunk_tiles[c]

            if hfin is not None:
                nc.vector.tensor_copy(hfin[:], state['h'][:, :])


def emit_head(nc, tc, w, h_bf, out_ext):
    with tc.tile_pool(name="head", bufs=1) as HP, \
         tc.tile_pool(name="headps", bufs=2, space="PSUM") as HPS:
        wo1 = HP.tile([P, KH * KH * P], MMDT, tag="wo1")
        for k in range(KH):
            nc.sync.dma_start(out=wo1[:, k * KH * P:(k + 1) * KH * P],
                                in_=w["Wo1"][k * P:(k + 1) * P, :])
        bo1 = HP.tile([P, KH], F32, tag="bo1")
        nc.sync.dma_start(out=bo1[:], in_=w["bo1"].rearrange("(m p) -> p m", p=P))
        wo2 = HP.tile([P, KH * OUT], MMDT, tag="wo2")
        for k in range(KH):
            nc.sync.dma_start(out=wo2[:, k * OUT:(k + 1) * OUT],
                                in_=w["Wo2"][k * P:(k + 1) * P, :])

        o1 = HP.tile([P, KH * BL], MMDT, tag="o1")
        ps1 = HPS.tile([P, KH * BL], F32, tag="o1")
        for m in range(KH):
            for k in range(KH):
                nc.tensor.matmul(ps1[:, m * BL:(m + 1) * BL],
                                 wo1[:, (k * KH + m) * P:(k * KH + m + 1) * P],
                                 h_bf[:, k * BL:(k + 1) * BL],
                                 start=(k == 0), stop=(k == KH - 1))
        for m in range(KH):
            nc.scalar.activation(o1[:, m * BL:(m + 1) * BL], ps1[:, m * BL:(m + 1) * BL],
                                 AF.Relu, bias=bo1[:, m:m + 1])
        ps2 = HPS.tile([OUT, BL], F32, tag="o2")
        for k in range(KH):
            nc.tensor.matmul(ps2[:], wo2[:, k * OUT:(k + 1) * OUT],
                             o1[:, k * BL:(k + 1) * BL],
                             start=(k == 0), stop=(k == KH - 1))
        osb = HP.tile([OUT, BL], F32, tag="osb")
        nc.vector.tensor_copy(osb[:], ps2[:])  # bo2 added host-side
        nc.sync.dma_start(out=out_ext[:], in_=osb[:])


_CACHED = {}


def _get_program(T_=T, C_=C):
    key = (T_, C_)
    if key not in _CACHED:
        _CACHED[key] = build_program(T_, C_)
    return _CACHED[key]


def make_in_maps(inputs):
    """Host-side prep: fp16 for everything a fp16 SBUF tile consumes (the
    device DMA must not cast), fp32 for biases."""
    x16 = np.ascontiguousarray(np.asarray(inputs["x"], dtype=np.float16))
    casted = {}
    for name in WEIGHT_NAMES:
        dt = np.float32 if name.split("_")[-1].startswith("b") else np.float16
        a = np.asarray(inputs[name], dtype=np.float64)
        # reservoir state is carried as s = r/LEAK (one fused stt op per
        # step); fold LEAK into every consumer of r exactly
        if name.endswith("Wres") or name.endswith("Wp"):
            a = a * LEAK
        elif name.endswith("Wg"):
            lin = IN if name.startswith("L0") else H
            a = a.copy()
            a[lin + H:] *= LEAK
        casted[name] = np.ascontiguousarray(a.astype(dt))
    in_maps = []
    for i in range(NCORES):
        m = {"x": np.ascontiguousarray(x16[i * BL:(i + 1) * BL])}
        m.update(casted)
        in_maps.append(m)
    return in_maps


def kernel(**inputs):
    inputs = {k: np.asarray(v) for k, v in inputs.items()}
    nc = _get_program()
    in_maps = make_in_maps(inputs)
    res = run_bass_kernel_spmd(nc, in_maps, list(range(NCORES)))
    outs = [res.results[i]["out"].T for i in range(NCORES)]  # (BL, OUT) each
    full = np.concatenate(outs, axis=0).astype(np.float32)
    return full + np.asarray(inputs["bo2"], dtype=np.float32)[None, :]


if __name__ == "__main__":
    # smoke test with random inputs
    rng = np.random.default_rng(0)
    nc = build_program(64, 32)
    print("built program OK")

